# revision 17
# baseline (speedup 1.0000x reference)
"""Trainium2 Bass kernel for nn_MeshAttentionBlock (B=4, V=1024, D=1024, H=16).

Sharding: 8 cores, no cross-core communication.  Core c handles batch
b = c // 2 and query-token half c % 2.  Inputs are token-reordered on
host so each core's 512 query tokens are rows 0:512 (attention is
permutation-equivariant over key order).

v2 redesign (PE-sequencer-bound baseline at 262us):
  * ada cond-MLP computed row-major (16 ap-512 matmuls per ada instead
    of 128 ap-1 matmuls) with a strided SBUF->SBUF DMA scatter to get
    the per-partition scale/shift layout.
  * edge bias moved off the DVE (was a 76us custom cubic) onto the PE:
    host sends per-head fp8 bias planes (32*table[e]); a DoubleRow
    matmul with a 256/..-scaled identity stationary accumulates them
    into the s psum pre-exp (256*32 = 8192 = 1/SIG exactly).
  * s = k^T q runs fp8 DoubleRow at half cycles via zero-plane padding:
    KM holds [Z, kT0, Z, kT1, ... kT7, Z] so head-even uses stationary
    positions (2m+1, 2m+2) = (kT, Z) and head-odd (2m, 2m+1) = (Z, kT);
    QM holds per-m [q(0:64); 0] / [0; q(64:128)] half-planes so the
    other head's rows are masked.  No extra copies, no negative strides.
  * LN1/LN2 transposes moved from the PE (96 transpose matmuls) to the
    DMA xbar (dma transpose, 14ns/16x128 tile), modulate runs
    post-transpose split across ACT/DVE.
  * weight/bias DMAs batched (one descriptor per 512-col block group)
    to cut HWDGE/SP-SEQ fixed overheads.
  * denominator: softmax normalize multiplies straight out of PSUM
    (recip row broadcast by a ones matmul as before, no ACT evac).
  fp8 quantization plan unchanged from v1: w_qkv/w_proj fp8*32,
  mlp w1 fp8*64 hi+residual, m2 bf16.
"""

import sys

for _p in ("/opt/trn_rl_repo",):
    if _p not in sys.path:
        sys.path.insert(0, _p)

import numpy as np

import concourse.bass as bass
import concourse.tile as tile
from concourse import bacc, mybir
from concourse.masks import make_identity

B, V, D = 4, 1024, 1024
H, HD = 16, 64
CD = 512
FF = 4096
EPS = 1e-5
P = 128
QH = 512  # query tokens per core

WS = 32.0    # w_qkv / w_proj host pre-scale (fp8 mantissa positioning)
W1S = 64.0   # mlp_w1 pre-scale
BS = 64.0    # edge-bias plane pre-scale; identity diag 128 = 8192/BS
BD = 128.0   # bias identity diagonal (128*64 = 8192 = 1/SIG)
LEX = float(np.log(16.0))  # exp overflow guard (softmax-invariant)
SIG = 0.125 / (WS * WS)

F32 = mybir.dt.float32
F32R = mybir.dt.float32r
BF16 = mybir.dt.bfloat16
FP8 = mybir.dt.float8e4
AF = mybir.ActivationFunctionType
ALU = mybir.AluOpType
DR = mybir.MatmulPerfMode.DoubleRow


def r(ap):
    """bitcast an fp32 AP to float32r for fast-rate PE matmuls."""
    return ap.bitcast(F32R)


def build_nc(mlp_fp8: str = "m1x2", dbg: bool = False):
    nc = bacc.Bacc("TRN2", target_bir_lowering=False)

    # ---- I/O ----
    x_full = nc.dram_tensor("x_full", [V, D], F32, kind="ExternalInput")
    bias_d = nc.dram_tensor("bias_d", [P, H, 8, QH], FP8, kind="ExternalInput")
    zero_d = nc.dram_tensor("zero_d", [P, 9, V], FP8, kind="ExternalInput")
    cond_c = nc.dram_tensor("cond_c", [P, 4], F32, kind="ExternalInput")
    ada1_w = nc.dram_tensor("ada1_w", [CD, 2 * D], BF16, kind="ExternalInput")
    ada2_w = nc.dram_tensor("ada2_w", [CD, 2 * D], BF16, kind="ExternalInput")
    # ada biases, host-transposed: [:, 0:8]=1+scale1 chunks, 8:16=shift1,
    # 16:24=1+scale2, 24:32=shift2
    abt_d = nc.dram_tensor("abt_d", [P, 32], F32, kind="ExternalInput")
    wqkv_p = nc.dram_tensor("wqkv_p", [P, 4, 2, 3 * D], FP8, kind="ExternalInput")
    wproj_p = nc.dram_tensor("wproj_p", [P, 4, 2, D], FP8, kind="ExternalInput")
    b_proj = nc.dram_tensor("b_proj", [1, D], F32, kind="ExternalInput")
    m1f8 = mlp_fp8 in ("m1", "m1x2")
    n1t = 2 if mlp_fp8 == "m1x2" else 1  # w1 fp8 terms (hi + residual)
    w1_p = nc.dram_tensor("w1_p", [P, 4, 2, FF], FP8, kind="ExternalInput")
    w1b_p = (
        nc.dram_tensor("w1b_p", [P, 4, 2, FF], FP8, kind="ExternalInput")
        if n1t == 2 else None
    )
    w2_p = nc.dram_tensor("w2_p", [P, 32, D], BF16, kind="ExternalInput")
    b1c = nc.dram_tensor("b1c", [P, FF // P], F32, kind="ExternalInput")
    mlp_b2 = nc.dram_tensor("mlp_b2", [1, D], F32, kind="ExternalInput")
    out_d = nc.dram_tensor("out", [QH, D], F32, kind="ExternalOutput")
    p_scr = nc.dram_tensor("p_scr", [2, 2 * D], F32, kind="Internal")
    dbg_d = {}
    if dbg:
        for nm, shp, dt_ in (
            ("d_hT", [P, 8, V], FP8), ("d_QM", [P, 8, 2, QH], FP8),
            ("d_KM", [P, 17, V], FP8), ("d_v", [P, 8, H, HD + 1], FP8),
            ("d_OT", [P, 8, QH], FP8),
            ("d_x2", [P, 4, D], F32), ("d_h2T", [P, 8, QH], FP8),
            ("d_sT", [P, 32], F32), ("d_ex0", [P, 8, QH], FP8),
            ("d_s0", [P, 2, QH], F32),
        ):
            dbg_d[nm] = nc.dram_tensor(nm, shp, dt_, kind="ExternalOutput")

    with tile.TileContext(nc) as tc:
        with (
            tc.tile_pool(name="persist", bufs=1) as pp,
            tc.tile_pool(name="w512", bufs=4) as wp512,
            tc.tile_pool(name="row", bufs=2) as rp,
            tc.tile_pool(name="att", bufs=4) as atp,
            tc.tile_pool(name="small", bufs=2) as smp,
            tc.tile_pool(name="mm", bufs=2, space="PSUM") as pmm,
        ):
            ident = pp.tile([P, P], BF16, tag="ident")
            make_identity(nc, ident)
            eps_t = pp.tile([P, 1], F32, tag="eps")
            nc.vector.memset(eps_t, EPS)
            ones_f = smp.tile([1, P], F32, tag="onesf", bufs=1, name="ones_f")
            nc.vector.memset(ones_f, 1.0)
            ones_t = pp.tile([1, P], F32R, tag="ones")
            nc.vector.tensor_copy(ones_t, ones_f)
            ones512 = pp.tile([1, 512], BF16, tag="o5", name="ones512")
            nc.vector.memset(ones512, 1.0)
            negl = pp.tile([P, 1], F32, tag="negl", name="negl")
            nc.vector.memset(negl, -LEX)
            # bias identity planes [Z, BD*I, Z]
            I3 = pp.tile([P, 3, P], FP8, tag="I3", name="I3")
            nc.vector.memset(I3[:, 0, :], 0.0)
            nc.vector.memset(I3[:, 2, :], 0.0)
            nc.vector.tensor_scalar_mul(I3[:, 1, :], ident, BD)

            # ---------- cond MLP (ada1 + ada2), row-major + DMA scatter ----
            condt = smp.tile([P, 4], F32, tag="condt")
            nc.sync.dma_start(out=condt, in_=cond_c[:, :])
            sig = smp.tile([P, 4], F32, tag="sig", name="sig")
            nc.scalar.activation(sig, condt, AF.Sigmoid)
            sc = pp.tile([P, 4], BF16, tag="sc")
            nc.vector.tensor_mul(sc, sig, condt)
            abt = pp.tile([P, 32], F32, tag="abt", name="abt")
            nc.sync.dma_start(out=abt, in_=abt_d[:, :])

            # sShT[:, 0:8]=1+scale1, [:,8:16]=shift1, [:,16:24]=1+scale2, ...
            sShT = pp.tile([P, 32], F32, tag="sShT", name="sShT")

            def _ada_block(ia, aw):
                p_row = smp.tile([1, 2 * D], F32, tag="prow", bufs=1,
                                 name="p_row")
                for half in range(2):
                    aw_sb = rp.tile([P, 4, D], BF16, tag="awsb", bufs=1,
                                    name="aw_sb")
                    nc.sync.dma_start(
                        out=aw_sb,
                        in_=aw[:, half * D : (half + 1) * D].rearrange(
                            "(k p) d -> p k d", p=P),
                    )
                    for gh in range(2):
                        g = half * 2 + gh
                        pg = pmm.tile([1, 512], F32, tag="mm", bufs=2,
                                      name="pg")
                        for k in range(4):
                            nc.tensor.matmul(
                                pg, sc[:, k : k + 1],
                                aw_sb[:, k, gh * 512 : (gh + 1) * 512],
                                start=(k == 0), stop=(k == 3),
                            )
                        nc.vector.tensor_copy(
                            p_row[:, g * 512 : (g + 1) * 512], pg)
                pT = smp.tile([P, 16], F32, tag="pT", bufs=2, name="pT")
                nc.sync.dma_start(out=p_scr[ia : ia + 1, :], in_=p_row)
                nc.sync.dma_start(
                    out=pT,
                    in_=p_scr[ia, :].rearrange("(j p) -> p j", p=P),
                )
                # +1 for the scale half is folded into the host-side bias
                nc.vector.scalar_tensor_tensor(
                    out=sShT[:, ia * 16 : ia * 16 + 16],
                    in0=pT, scalar=1.0, in1=abt[:, ia * 16 : ia * 16 + 16],
                    op0=ALU.mult, op1=ALU.add,
                )

            # ---------- LN1 (stats+norm in row space, DMA-xbar transpose) ----
            hT_all = pp.tile([P, 8, V], FP8, tag="hT", name="hT_all")
            # xnT shares the "big16" ring with the two gT halves (16KB each):
            # gT's allocations evict xnT after the LN1 modulates are done.
            xnT = rp.tile([P, 8, V], BF16, tag="big16", bufs=2, name="xnT")
            x_sb = pp.tile([P, 4, D], F32, tag="xsb", name="x_sb")

            def _ln_stats(x_in, mv8, i):
                stats = smp.tile([P, 2, 6], F32, tag="stats", name="stats")
                xv = x_in.rearrange("p (s f) -> p s f", s=2)
                for s in range(2):
                    nc.vector.bn_stats(stats[:, s, :], xv[:, s, :])
                nc.vector.bn_aggr(mv8[:, i, :], stats)

            def _ln_norm(x_in, xn_out, mv8, rstd8, i):
                nc.vector.tensor_scalar(
                    out=xn_out, in0=x_in,
                    scalar1=mv8[:, i, 0:1], scalar2=rstd8[:, i : i + 1],
                    op0=ALU.subtract, op1=ALU.mult,
                )

            mv8_1 = smp.tile([P, 8, 2], F32, tag="mv81", bufs=1, name="mv8_1")
            rstd8_1 = smp.tile([P, 8], F32, tag="rs81", bufs=1, name="rstd8_1")
            for i in range(8):
                if i < 4:
                    xt = x_sb[:, i, :]
                else:
                    xt = rp.tile([P, D], F32, tag="row4", bufs=2, name="xt")
                nc.sync.dma_start(out=xt, in_=x_full[i * P : (i + 1) * P, :])
                _ln_stats(xt, mv8_1, i)
                sd = smp.tile([P, 1], F32, tag="sd", bufs=4, name="sd")
                nc.scalar.activation(sd, mv8_1[:, i, 1:2], AF.Sqrt,
                                     bias=eps_t)
                nc.vector.reciprocal(rstd8_1[:, i : i + 1], sd)
                xn_i = rp.tile([P, D], BF16, tag="xn", bufs=4, name="xn_i")
                _ln_norm(xt, xn_i, mv8_1, rstd8_1, i)
                nc.sync.dma_start(
                    out=xnT[:, :, i * P : (i + 1) * P], in_=xn_i,
                    transpose=True,
                )

            _ada_block(0, ada1_w)

            # modulate: hT = xnT * sT + shT (fp8); split ACT/DVE
            for k in range(8):
                if k % 2 == 0:
                    nc.scalar.activation(
                        hT_all[:, k, :], xnT[:, k, :], AF.Identity,
                        bias=sShT[:, 8 + k : 9 + k], scale=sShT[:, k : k + 1],
                    )
                else:
                    nc.vector.tensor_scalar(
                        out=hT_all[:, k, :], in0=xnT[:, k, :],
                        scalar1=sShT[:, k : k + 1],
                        scalar2=sShT[:, 8 + k : 9 + k],
                        op0=ALU.mult, op1=ALU.add,
                    )

            if dbg:
                nc.sync.dma_start(out=dbg_d["d_hT"][:], in_=hT_all[:])
                nc.sync.dma_start(out=dbg_d["d_sT"][:], in_=sShT[:])

            # ---------- QKV (fp8 DoubleRow, K=256 per matmul) ----------
            # KM: [Z, kT0, Z, kT1, ..., kT7, Z] zero/key planes for the
            # DR-padded s-matmul; QM: per-m [q(0:64);0] / [0;q(64:128)].
            KM = pp.tile([P, 17, V], FP8, tag="KM", name="KM")
            nc.sync.dma_start(
                out=KM[:, 0:16, :].rearrange("p (a b) v -> p a b v",
                                             b=2)[:, :, 0, :],
                in_=zero_d[:, 0:8, :],
            )
            nc.sync.dma_start(out=KM[:, 16, :], in_=zero_d[:, 8, :])
            QM = pp.tile([P, 8, 2, QH], FP8, tag="QM", name="QM")
            nc.sync.dma_start(
                out=QM[64:128, :, 0, :],
                in_=zero_d[64:128, 0:4, :].rearrange("p a (b q) -> p (a b) q",
                                                     q=QH),
            )
            nc.sync.dma_start(
                out=QM[0:64, :, 1, :],
                in_=zero_d[0:64, 0:4, :].rearrange("p a (b q) -> p (a b) q",
                                                   q=QH),
            )
            v_all = pp.tile([P, 8, H, HD + 1], FP8, tag="v", name="v_all")
            nc.vector.memset(v_all[:, :, :, HD : HD + 1], 1.0)

            def _v_block(n):
                wv = wp512.tile([P, 4, 2, 512], FP8, tag="wld", bufs=4,
                                name="wv")
                nc.sync.dma_start(
                    out=wv,
                    in_=wqkv_p[:, :, :, 2 * D + n * 512 : 2 * D + (n + 1) * 512],
                )
                for i2 in range(4):
                    ps = pmm.tile([P, 2, 512], F32, tag="s2", bufs=2,
                                  name="v_ps")
                    for j in range(2):
                        i = 2 * i2 + j
                        for c in range(4):
                            nc.tensor.matmul(
                                ps[:, j, :],
                                hT_all[:, 2 * c : 2 * c + 2,
                                       i * P : (i + 1) * P],
                                wv[:, c, :, :],
                                start=(c == 0), stop=(c == 3), perf_mode=DR,
                            )
                    if i2 % 2 == 0:
                        nc.vector.tensor_copy(
                            v_all[:, 2 * i2 : 2 * i2 + 2,
                                  n * 8 : (n + 1) * 8, 0:HD],
                            ps.rearrange("p i (h d) -> p i h d", d=HD),
                        )
                    else:
                        nc.scalar.activation(
                            v_all[:, 2 * i2 : 2 * i2 + 2,
                                  n * 8 : (n + 1) * 8, 0:HD],
                            ps.rearrange("p i (h d) -> p i h d", d=HD),
                            AF.Identity,
                        )

            def _kq_block(m4):
                wk = wp512.tile([P, 4, 2, 512], FP8, tag="wld", bufs=4,
                                name="wk")
                nc.sync.dma_start(
                    out=wk,
                    in_=wqkv_p[:, :, :, D + m4 * 512 : D + (m4 + 1) * 512],
                )
                for mi in range(4):
                    m = m4 * 4 + mi
                    for n in range(2):
                        ps = pmm.tile([P, 512], F32, tag="mm", name="k_ps")
                        for c in range(4):
                            nc.tensor.matmul(
                                ps,
                                wk[:, c, :, mi * P : (mi + 1) * P],
                                hT_all[:, 2 * c : 2 * c + 2,
                                       n * 512 : (n + 1) * 512],
                                start=(c == 0), stop=(c == 3), perf_mode=DR,
                            )
                        nc.scalar.activation(
                            KM[:, 2 * m + 1, n * 512 : (n + 1) * 512], ps,
                            AF.Identity,
                        )
                wq = wp512.tile([P, 4, 2, 512], FP8, tag="wld", bufs=4,
                                name="wq")
                nc.sync.dma_start(
                    out=wq,
                    in_=wqkv_p[:, :, :, m4 * 512 : (m4 + 1) * 512],
                )
                for mi in range(4):
                    m = m4 * 4 + mi
                    ps = pmm.tile([P, QH], F32, tag="mm", name="q_ps")
                    for c in range(4):
                        nc.tensor.matmul(
                            ps, wq[:, c, :, mi * P : (mi + 1) * P],
                            hT_all[:, 2 * c : 2 * c + 2, 0:QH],
                            start=(c == 0), stop=(c == 3), perf_mode=DR,
                        )
                    nc.scalar.activation(QM[0:64, m, 0, :], ps[0:64, :],
                                         AF.Identity)
                    nc.vector.tensor_copy(QM[64:128, m, 1, :], ps[64:128, :])

            _v_block(0)
            _kq_block(0)
            _v_block(1)
            _kq_block(1)

            if dbg:
                nc.sync.dma_start(out=dbg_d["d_QM"][:], in_=QM[:])
                nc.sync.dma_start(out=dbg_d["d_KM"][:], in_=KM[:])
                nc.sync.dma_start(out=dbg_d["d_v"][:], in_=v_all[:])

            # ---------- attention (16 heads, full 512-query width) ----------
            # s psum holds 1024*s_true + 8192*table[e]; exp scale SIG,
            # bias -ln16 (overflow guard; softmax-invariant), fp8 out;
            # av DoubleRow over paired key chunks with a ones row for the
            # softmax denominator.
            OT_all = pp.tile([P, 8, QH], FP8, tag="OT", name="OT_all")

            def _finish_head(pend):
                h, ex, ot_ps = pend
                m, lo = h // 2, (h % 2) * HD
                for g in range(4):
                    nc.tensor.matmul(
                        ot_ps, v_all[:, 2 * g : 2 * g + 2, h, :],
                        ex[:, 2 * g : 2 * g + 2, :],
                        start=(g == 0), stop=(g == 3), perf_mode=DR,
                    )
                recip = smp.tile([1, QH], F32R, tag="recip", bufs=2,
                                 name="recip")
                with nc.allow_low_precision(reason="f32r recip bcast"):
                    nc.vector.reciprocal(recip, ot_ps[HD : HD + 1, :])
                rc_ps = pmm.tile([HD, QH], F32, tag="mm", name="rc_ps")
                nc.tensor.matmul(
                    rc_ps, r(ones_t[:, 0:HD]), r(recip), start=True, stop=True
                )
                recb = atp.tile([HD, QH], F32, tag="recb", bufs=2,
                                name="recb")
                nc.scalar.activation(recb, rc_ps, AF.Identity)
                nc.vector.tensor_mul(
                    OT_all[lo : lo + HD, m, :], ot_ps[0:HD, :], recb
                )

            def _bias_pf(h):
                bt = atp.tile([P, 8, QH], FP8, tag="bpl", bufs=2,
                              name=f"bias{h}")
                nc.sync.dma_start(out=bt, in_=bias_d[:, h, :, :])
                return bt

            bias_t = {h: None for h in range(H)}
            bias_t[0] = _bias_pf(0)
            pend = None
            for h in range(H):
                m, eo = h // 2, h % 2
                st_lo = 2 * m + 1 - eo  # stationary start plane in KM
                bt = bias_t[h]
                ex = atp.tile([P, 8, QH], FP8, tag="ex", bufs=2, name="ex")
                ot_ps = pmm.tile([HD + 1, QH], F32, tag="ot", bufs=2,
                                 name="ot_ps")
                for g in range(4):
                    s2 = pmm.tile([P, 2, QH], F32, tag="s2", bufs=2,
                                  name="s2")
                    for j in range(2):
                        kc = 2 * g + j
                        nc.tensor.matmul(
                            s2[:, j, :],
                            KM[:, st_lo : st_lo + 2, kc * P : (kc + 1) * P],
                            QM[:, m, :, :],
                            start=True, stop=False, perf_mode=DR,
                        )
                        # edge-bias accumulate: (BD*I) @ bias_plane
                        if kc < 7:
                            nc.tensor.matmul(
                                s2[:, j, :], I3[:, 1:3, :],
                                bt[:, kc : kc + 2, :],
                                start=False, stop=True, perf_mode=DR,
                            )
                        else:
                            nc.tensor.matmul(
                                s2[:, j, :], I3[:, 0:2, :],
                                bt[:, 6:8, :],
                                start=False, stop=True, perf_mode=DR,
                            )
                    nc.scalar.activation(
                        ex[:, 2 * g : 2 * g + 2, :].rearrange(
                            "p a b -> p (a b)"),
                        s2.rearrange("p a b -> p (a b)"),
                        AF.Exp, bias=negl, scale=SIG,
                    )
                    if dbg and h == 0 and g == 0:
                        sdmp = atp.tile([P, 2, QH], F32, tag="bpl", bufs=2,
                                        name="sdmp")
                        nc.vector.tensor_copy(sdmp, s2)
                        nc.sync.dma_start(out=dbg_d["d_s0"][:], in_=sdmp)
                    if g == 0 and pend is not None:
                        _finish_head(pend)
                        pend = None
                if dbg and h == 0:
                    nc.sync.dma_start(out=dbg_d["d_ex0"][:], in_=ex)
                pend = (h, ex, ot_ps)
                if h + 1 < H:
                    bias_t[h + 1] = _bias_pf(h + 1)
                if False:
                    _v_block(1)
                    _kq_block(1)
                if h == 8:
                    _ada_block(1, ada2_w)
            _finish_head(pend)

            # ---------- proj (DR) + residual (in place) + LN2 ----------
            bp_r = pp.tile([1, D], BF16, tag="bpr", name="bp_r")
            bpf = rp.tile([1, D], F32, tag="row4", bufs=2, name="bpf")
            nc.sync.dma_start(out=bpf, in_=b_proj[0:1, :])
            nc.vector.tensor_scalar_mul(bp_r, bpf, WS * WS)
            x2_all = x_sb
            h2T_all = pp.tile([P, 8, QH], FP8, tag="h2T", name="h2T_all")
            xnT2 = pp.tile([P, 8, QH], BF16, tag="hT", name="xnT2")
            mv8_2 = smp.tile([P, 4, 2], F32, tag="mv82", bufs=1, name="mv8_2")
            rstd8_2 = smp.tile([P, 4], F32, tag="rs82", bufs=1,
                               name="rstd8_2")
            wpn = []
            for n in range(2):
                wp = wp512.tile([P, 4, 2, 512], FP8, tag="wld", bufs=4,
                                name="wp")
                nc.sync.dma_start(
                    out=wp, in_=wproj_p[:, :, :, n * 512 : (n + 1) * 512])
                wpn.append(wp)
            for mm_ in range(4):
                for n in range(2):
                    ps = pmm.tile([P, 512], F32, tag="mm", name="pr_ps")
                    for c in range(4):
                        nc.tensor.matmul(
                            ps,
                            OT_all[:, 2 * c : 2 * c + 2,
                                   mm_ * P : (mm_ + 1) * P],
                            wpn[n][:, c, :, :],
                            start=(c == 0), stop=False, perf_mode=DR,
                        )
                    nc.tensor.matmul(
                        ps, ones512[:, 0:P],
                        bp_r[0:1, n * 512 : (n + 1) * 512],
                        start=False, stop=True,
                    )
                    nc.vector.scalar_tensor_tensor(
                        out=x2_all[:, mm_, n * 512 : (n + 1) * 512],
                        in0=ps, scalar=1.0 / (WS * WS),
                        in1=x2_all[:, mm_, n * 512 : (n + 1) * 512],
                        op0=ALU.mult, op1=ALU.add,
                    )
                i = mm_
                _ln_stats(x2_all[:, i, :], mv8_2, i)
                sd = smp.tile([P, 1], F32, tag="sd", bufs=4, name="sd2")
                nc.scalar.activation(sd, mv8_2[:, i, 1:2], AF.Sqrt,
                                     bias=eps_t)
                nc.vector.reciprocal(rstd8_2[:, i : i + 1], sd)
                xn_i = rp.tile([P, D], BF16, tag="xn", bufs=4, name="xn2_i")
                _ln_norm(x2_all[:, i, :], xn_i, mv8_2, rstd8_2, i)
                nc.sync.dma_start(
                    out=xnT2[:, :, i * P : (i + 1) * P], in_=xn_i,
                    transpose=True,
                )
            for k in range(8):
                if k % 2 == 0:
                    nc.scalar.activation(
                        h2T_all[:, k, :], xnT2[:, k, :], AF.Identity,
                        bias=sShT[:, 24 + k : 25 + k],
                        scale=sShT[:, 16 + k : 17 + k],
                    )
                else:
                    nc.vector.tensor_scalar(
                        out=h2T_all[:, k, :], in0=xnT2[:, k, :],
                        scalar1=sShT[:, 16 + k : 17 + k],
                        scalar2=sShT[:, 24 + k : 25 + k],
                        op0=ALU.mult, op1=ALU.add,
                    )

            if dbg:
                nc.sync.dma_start(out=dbg_d["d_OT"][:], in_=OT_all[:])
                nc.sync.dma_start(out=dbg_d["d_x2"][:], in_=x_sb[:])
                nc.sync.dma_start(out=dbg_d["d_h2T"][:], in_=h2T_all[:])

            # ---------- MLP (w1 as hi+residual fp8 terms; m2 n=0
            # interleaved into the m1 f4 loop) ----------
            b1_sb = pp.tile([P, FF // P], F32, tag="b1sb")
            nc.sync.dma_start(out=b1_sb, in_=b1c[:, :])
            b2_r = pp.tile([1, D], BF16, tag="b2r", name="b2_r")
            b2f = rp.tile([1, D], F32, tag="row4", bufs=2, name="b2f")
            nc.sync.dma_start(out=b2f, in_=mlp_b2[0:1, :])
            nc.vector.tensor_scalar_mul(b2_r, b2f, 1.0)
            gT_a = rp.tile([P, 16, QH], BF16, tag="big16", bufs=2,
                           name="gT_a")
            gT_b = rp.tile([P, 16, QH], BF16, tag="big16", bufs=2,
                           name="gT_b")

            def gT(f):
                return gT_a[:, f, :] if f < 16 else gT_b[:, f - 16, :]

            def gTs(fc, sl):
                t = gT_a if fc < 16 else gT_b
                f = fc if fc < 16 else fc - 16
                return t[:, f : f + 1, sl]

            def _m2_mms(n, f4, ps_acc):
                wt = wp512.tile([P, 4, 512], BF16, tag="wld2", bufs=3,
                                name="w2t")
                nc.sync.dma_start(
                    out=wt,
                    in_=w2_p[:, f4 * 4 : (f4 + 1) * 4,
                             n * 512 : (n + 1) * 512],
                )
                for fi in range(4):
                    fc = f4 * 4 + fi
                    for mm_ in range(4):
                        nc.tensor.matmul(
                            ps_acc[mm_],
                            gTs(fc, slice(mm_ * P, (mm_ + 1) * P)),
                            wt[:, fi : fi + 1, :],
                            start=(fc == 0), stop=False,
                        )
                        if fc == 31:
                            nc.tensor.matmul(
                                ps_acc[mm_],
                                ones512[:, 0:P],
                                b2_r[0:1, n * 512 : (n + 1) * 512],
                                start=False, stop=True,
                            )

            def _m2_evac(n, ps_acc):
                for mm_ in range(4):
                    ot = rp.tile([P, 512], F32, tag="s512", bufs=2, name="ot")
                    nc.vector.tensor_add(
                        ot, ps_acc[mm_],
                        x2_all[:, mm_, n * 512 : (n + 1) * 512],
                    )
                    nc.sync.dma_start(
                        out=out_d[mm_ * P : (mm_ + 1) * P,
                                  n * 512 : (n + 1) * 512],
                        in_=ot,
                    )

            def _mk_acc():
                a2_ = [
                    pmm.tile([P, 2, 512], F32, tag="s2", bufs=2,
                             name="m2acc")
                    for _ in range(2)
                ]
                return [a2_[j][:, o, :] for j in range(2) for o in range(2)]

            ps_acc0 = _mk_acc()
            for f4 in range(8):
                w1s = []
                for term in range(n1t):
                    w1d = w1_p if term == 0 else w1b_p
                    wt = wp512.tile([P, 4, 2, 512], FP8, tag="wld1",
                                    bufs=3, name="w1t")
                    nc.sync.dma_start(
                        out=wt, in_=w1d[:, :, :, f4 * 512 : (f4 + 1) * 512]
                    )
                    w1s.append(wt)
                for fi in range(4):
                    f = f4 * 4 + fi
                    ps = pmm.tile([P, QH], F32, tag="mm", name="m1_ps")
                    for t_ in range(n1t * 4):
                        c = t_ % 4
                        nc.tensor.matmul(
                            ps, w1s[t_ // 4][:, c, :, fi * P : (fi + 1) * P],
                            h2T_all[:, 2 * c : 2 * c + 2, :],
                            start=(t_ == 0), stop=(t_ == n1t * 4 - 1),
                            perf_mode=DR,
                        )
                    nc.scalar.activation(
                        gT(f), ps, AF.Gelu,
                        bias=b1_sb[:, f : f + 1],
                        scale=1.0 / W1S,
                    )
                _m2_mms(0, f4, ps_acc0)
            _m2_evac(0, ps_acc0)
            ps_acc1 = _mk_acc()
            for f4 in range(8):
                _m2_mms(1, f4, ps_acc1)
            _m2_evac(1, ps_acc1)

    nc.compile()
    return nc


_BUILD_CACHE = {}
MLP_FP8 = "m1x2"


def _get_nc(mlp_fp8=None, dbg=False):
    if mlp_fp8 is None:
        mlp_fp8 = MLP_FP8
    key = (mlp_fp8, dbg)
    if key not in _BUILD_CACHE:
        _BUILD_CACHE[key] = build_nc(mlp_fp8, dbg)
    return _BUILD_CACHE[key]


def _pack_dr(w, scale, dt, ki=2):
    """[K, N] -> [128, K//(128*ki), ki, N] layout, contraction index
    k = chunk_outer*128*ki + o*128 + p."""
    K, N = np.asarray(w).shape
    return np.ascontiguousarray(
        (np.asarray(w, np.float32) * scale)
        .reshape(K // (P * ki), ki, P, N)
        .transpose(2, 0, 1, 3)
        .astype(dt)
    )


def make_in_maps(inputs, mlp_fp8=None):
    import ml_dtypes

    if mlp_fp8 is None:
        mlp_fp8 = MLP_FP8
    fp8 = ml_dtypes.float8_e4m3
    bf16 = ml_dtypes.bfloat16
    x = np.asarray(inputs["x"], np.float32)
    cond = np.asarray(inputs["cond"], np.float32)
    e = np.asarray(inputs["edge_index"], np.int32)
    tab = np.asarray(inputs["edge_table"], np.float32)  # [4, H]

    def _abt(b):
        a = np.asarray(b, np.float32).reshape(16, P).T.copy()
        a[:, 0:8] += 1.0  # 1+scale folded here
        return a

    abt_all = np.ascontiguousarray(
        np.concatenate([_abt(inputs["ada1_b"]), _abt(inputs["ada2_b"])],
                       axis=1)
    )

    shared = {
        "ada1_w": np.asarray(inputs["ada1_w"], np.float32).astype(bf16),
        "ada2_w": np.asarray(inputs["ada2_w"], np.float32).astype(bf16),
        "abt_d": abt_all,
        "wqkv_p": _pack_dr(inputs["w_qkv"], WS, fp8),
        "wproj_p": _pack_dr(inputs["w_proj"], WS, fp8),
        "b_proj": np.asarray(inputs["b_proj"], np.float32).reshape(1, D),
        "w1_p": _pack_dr(inputs["mlp_w1"], W1S, fp8, 2),
        "w2_p": np.ascontiguousarray(
            np.asarray(inputs["mlp_w2"], np.float32)
            .reshape(32, P, D).transpose(1, 0, 2).astype(bf16)
        ),
        "b1c": np.ascontiguousarray(
            np.asarray(inputs["mlp_b1"], np.float32).reshape(FF // P, P).T
        ),
        "mlp_b2": np.asarray(inputs["mlp_b2"], np.float32).reshape(1, D),
        "zero_d": np.zeros((P, 9, V), fp8),
    }
    if mlp_fp8 == "m1x2":
        w1s_ = np.asarray(inputs["mlp_w1"], np.float32) * W1S
        w1hi = w1s_.astype(fp8)
        shared["w1b_p"] = _pack_dr(w1s_ - w1hi.astype(np.float32), 1.0, fp8, 2)

    # bias planes: [P, H, 8, QH]; plane[p, h, kc, q] = BS*tab[e[key, q], h]
    # with key = kc*128+p (token-permuted per core).
    tab_s = (BS * tab).astype(np.float32)  # [4, H]
    in_maps = []
    idx = np.arange(V)
    swap = np.r_[QH:V, 0:QH]
    for c in range(8):
        b, half = c // 2, c % 2
        perm = swap if half else idx
        xb = np.ascontiguousarray(x[b][perm])
        eb = e[b][np.ix_(perm[:QH], perm)]  # [QH(q), V(key)]
        eT = eb.T  # [V(key), QH(q)]
        # bias_d[p, h, kc, q] = tab_s[eT[kc*128+p, q], h]
        bias = tab_s[eT]  # [V, QH, H] f32
        bias = (
            bias.reshape(8, P, QH, H).transpose(1, 3, 0, 2).astype(fp8)
        )  # [P, H, 8, QH]
        cc = np.ascontiguousarray(cond[b].reshape(4, P).T)
        in_maps.append(
            {"x_full": xb, "bias_d": np.ascontiguousarray(bias),
             "cond_c": cc, **shared}
        )
    return in_maps


def kernel(**inputs):
    from concourse import bass_utils

    nc = _get_nc()
    in_maps = make_in_maps(inputs)
    res = bass_utils.run_bass_kernel_spmd(nc, in_maps, core_ids=list(range(8)))
    out = np.empty((B, V, D), np.float32)
    for c in range(8):
        b, half = c // 2, c % 2
        out[b, half * QH : (half + 1) * QH] = res.results[c]["out"]
    return out
